# revision 82
# baseline (speedup 1.0000x reference)
"""Collaborative RNN (GRU-style user-state scan + big vocab projection) on 8 trn2 cores.

Strategy
--------
Data-parallel over batch: core c owns batch rows [4c, 4c+4) (512 (b,t) pairs).
Each core runs the scan for its rows and computes logits for its 512 output
rows over the FULL vocab -> [512, 30001]; host concatenates.

The scan is restructured by dependency *levels*: pair (b,t) depends only on the
previous occurrence of the same user in the same batch row.  With U=256 users
and S=128 steps most users appear 0-2 times, so the 128-step serial scan
collapses into ~5 fully-batched levels.  Level 0 (first occurrences) needs no
hidden-state input at all when h0 == 0 (the graded case).

Per-core index structure is passed as *data* (index vectors; one-hot
gather/scatter matrices are generated on device via iota + is_equal) so a
single SPMD program runs on all 8 cores.  The program itself only depends on
global level sizes.

Layouts: "T" tiles are [H=128 partitions, pairs in free dim]; "nat" tiles are
[pairs in partitions, H in free dim].  The gather matmul contracts pair chunks
of the natural state against on-device one-hots and yields h_prev directly in
transposed layout; embedding rows are accumulated into the r/z/c PSUMs with
transpose-matmuls, so the only explicit transpose per level is h_new back to
natural for the scatter matmul.
"""

import sys
import types

import ml_dtypes
import numpy as np

# ---------------------------------------------------------------- constants
B, S, U, H, V = 32, 128, 256, 128, 30001
NC = 8
R = B // NC  # batch rows per core
N = R * S  # 512 output rows (pairs) per core
H2 = 2 * H
P = 128
NCH = N // P  # pair chunks per core
WS_CHUNK = 8192  # ws free-dim tile width
STG_CHUNK = 8192  # staging tile width (16KB bf16 row-segments per DMA)
PS_N = 1024  # PSUM tile width (2 banks) — one copy op per PSUM tile
MM_N = 512  # moving free dim per matmul

TRACE = False  # set by test.py for profiling runs
_LAST_RESULTS = {}  # test.py reads exec_time_ns etc. from here


def _install_ntff_hook():
    """Register the axon NTFF profiling hook (antenv.axon_hooks is a stub in
    this container).  Harmless if the .so lacks the profiling symbols."""
    try:
        import antenv

        if getattr(antenv, "axon_hooks", None) is not None:
            return
        mod = types.ModuleType("antenv.axon_hooks")
        mod._hook = None
        mod.set_axon_ntff_profile_hook = lambda h: setattr(mod, "_hook", h)
        mod.get_axon_ntff_profile_hook = lambda: mod._hook
        sys.modules["antenv.axon_hooks"] = mod
        antenv.axon_hooks = mod
        from trn_agent_boot.trn_boot import _ntff_profile_via_ctypes

        hook = _ntff_profile_via_ctypes("/opt/axon/libaxon_pjrt.so")
        if hook is not None:
            mod.set_axon_ntff_profile_hook(hook)
    except Exception:
        pass


# ---------------------------------------------------------------- host prep
def _pack_layout(kmax, nk):
    """Column offsets of the single packed [P, TOT] f32 scan-input tensor
    (items + per-level item indices; int32 bitcast to f32)."""
    off = {}
    o = 0
    off["items"] = o
    o += NCH
    for k in range(1, kmax):
        J = (nk[k] + P - 1) // P
        off[f"idx{k}"] = o
        o += J
    return off, o


def _pad4(n):
    return -(-n // 4) * 4


def _allT_layout(kmax, nk):
    """Column offsets of each level's output block inside the transposed
    all-states tile allT [H, 512 + sum(pad4(nk))]."""
    aoff = {0: 0}
    o = N
    for k in range(1, kmax):
        aoff[k] = o
        o += _pad4(nk[k])
    return aoff, o


def _ag_layout(kmax, nk):
    """ap_gather index columns (int16, 16-partition-wrapped): one entry per
    level's h_prev fetch plus one per final hT chunk.  Entries are aligned
    to 4 int16 columns — a 128-index gather with an odd column offset
    mis-gathers one 16-index group (observed on hardware)."""
    off = {}
    o = 0
    for k in range(1, kmax):
        nj4 = _pad4(nk[k])
        w = -(-nj4 // 16)
        off[("lvl", k)] = (o, w, nj4)
        o += _pad4(w)
    for c in range(NCH):
        off[("fin", c)] = (o, P // 16, P)
        o += P // 16
    return off, o


def _fold(a, cols):
    """[cols*128] -> [128, cols] with column j = slice j*128:(j+1)*128."""
    return np.ascontiguousarray(a.reshape(cols, P).T)


def _levels_for_core(users_c):
    """occ/prev per flat pair index (p = r*S + t, natural order)."""
    occ = np.zeros(N, np.int32)
    prev = np.full(N, -1, np.int32)
    for r in range(R):
        seen_cnt = {}
        seen_last = {}
        row = users_c[r]
        for t in range(S):
            u = int(row[t])
            p = r * S + t
            occ[p] = seen_cnt.get(u, 0)
            prev[p] = seen_last.get(u, -1)
            seen_cnt[u] = occ[p] + 1
            seen_last[u] = p
    return occ, prev


def _build_core_data(users, items, h0, with_h0):
    """Per-core level structure + global padded sizes."""
    cores = []
    kmax = 1
    for c in range(NC):
        occ, prev = _levels_for_core(users[c * R : (c + 1) * R])
        cores.append((occ, prev))
        kmax = max(kmax, int(occ.max()) + 1)

    nk = [0] * kmax
    for occ, _ in cores:
        for k in range(1, kmax):
            nk[k] = max(nk[k], int((occ == k).sum()))
    nk = [max(2, n) if k > 0 else 0 for k, n in enumerate(nk)]
    AGOFF, AGW = _ag_layout(kmax, nk)
    AOFF, AW = _allT_layout(kmax, nk)

    def wrap16(vals, o, w, ag):
        """Place vals (len<=w*16) into ag cols [o, o+w) in the 16-partition-
        wrapped layout each gpsimd core expects."""
        for p in range(P):
            for cc in range(w):
                i = cc * 16 + (p % 16)
                if i < len(vals) and vals[i] >= 0:
                    ag[p, o + cc] = np.int16(vals[i])

    per_core = []
    for c in range(NC):
        occ, prev = cores[c]
        items_c = items[c * R : (c + 1) * R].reshape(-1).astype(np.int32)
        d = {}
        if with_h0:
            users_c = users[c * R : (c + 1) * R].reshape(-1).astype(np.int32)
            local_r = np.repeat(np.arange(R, dtype=np.int32), S)
            d["h0_idx"] = _fold(local_r * U + users_c, NCH)
            d["h0c"] = np.ascontiguousarray(
                h0[c * R : (c + 1) * R].reshape(R * U, H), dtype=np.float32
            )
        parts = [_fold(items_c, NCH).view(np.float32)]
        ag = np.zeros((P, AGW), np.int16)
        # final position (column in allT) of each pair's output
        fpos = np.arange(N, dtype=np.int64)
        prev_pos = {}  # pair id -> allT column of its h
        for p in range(N):
            prev_pos[p] = p  # level-0 columns
        for k in range(1, kmax):
            n = nk[k]
            J = (n + P - 1) // P
            pk = np.nonzero(occ == k)[0]
            idx_v = np.zeros(J * P, np.int32)
            m = len(pk)
            idx_v[:m] = items_c[pk]
            parts.append(_fold(idx_v, J).view(np.float32))
            # h_prev fetch indices: allT column of the predecessor's output
            gv = np.zeros(n, np.int64)
            for i, p in enumerate(pk):
                gv[i] = prev_pos[int(prev[p])]
            o, w, nj4 = AGOFF[("lvl", k)]
            wrap16(gv[: min(n, nj4)], o, w, ag)
            for i, p in enumerate(pk):
                col = AOFF[k] + i
                prev_pos[int(p)] = col
                fpos[p] = col
        for cc in range(NCH):
            o, w, _ = AGOFF[("fin", cc)]
            wrap16(fpos[cc * P : (cc + 1) * P], o, w, ag)
        d["pack"] = np.ascontiguousarray(np.concatenate(parts, axis=1))
        d["agidx"] = ag
        per_core.append(d)
    return per_core, kmax, nk


# ---------------------------------------------------------------- device build
def _build_program(kmax, nk, with_h0):
    import concourse.bacc as bacc
    import concourse.mybir as mybir
    import concourse.tile as tile
    from concourse import bass
    from concourse.masks import make_identity

    f32 = mybir.dt.float32
    bf16 = mybir.dt.bfloat16
    f32r = mybir.dt.float32r
    i32 = mybir.dt.int32
    AF = mybir.ActivationFunctionType
    OP = mybir.AluOpType

    nc = bacc.Bacc(None, target_bir_lowering=False)

    i16 = mybir.dt.int16

    # ---- DRAM I/O  (biases are folded into P_cat on the host)
    OFF, TOT = _pack_layout(kmax, nk)
    AGOFF, AGW = _ag_layout(kmax, nk)
    AOFF, AW = _allT_layout(kmax, nk)
    pack = nc.dram_tensor("pack", [P, TOT], f32, kind="ExternalInput")
    agidx = nc.dram_tensor("agidx", [P, AGW], i16, kind="ExternalInput")
    P_cat = nc.dram_tensor("P_cat", [V, H2 + H], f32, kind="ExternalInput")
    P_zc = nc.dram_tensor("P_zc", [V, H2], f32, kind="ExternalInput")
    W_ru = nc.dram_tensor("W_ru", [H, H2], f32, kind="ExternalInput")
    W_c = nc.dram_tensor("W_c", [H, H], f32, kind="ExternalInput")
    ws = nc.dram_tensor("ws", [H, V], bf16, kind="ExternalInput")
    logits = nc.dram_tensor("logits", [N, V], bf16, kind="ExternalOutput")
    if with_h0:
        h0_idx = nc.dram_tensor("h0_idx", [P, NCH], i32, kind="ExternalInput")
        h0c = nc.dram_tensor("h0c", [R * U, H], f32, kind="ExternalInput")

    ws_splits = [(v0, min(WS_CHUNK, V - v0)) for v0 in range(0, V, WS_CHUNK)]

    with tile.TileContext(nc) as tc, tc.tile_pool(name="const", bufs=1) as cpool:
        with (
            tc.tile_pool(name="scan", bufs=2) as spool,
            tc.tile_pool(name="scan_ps", bufs=1, space="PSUM") as spsum,
        ):
            # ---- emission order matters: each engine queue executes in the
            # scheduled (roughly program) order, so the scan's critical-path
            # ops are emitted FIRST and bulk work (ws load + bf16 casts) LAST.

            # ONE packed DMA for every small scan input (items + per-level
            # invm/idx/pk/prev): separate tiny DMAs each cost ~0.6us of queue
            # issue time and were landing behind the bulk ws transfers.
            pack_sb = cpool.tile([P, TOT], f32, tag="pack_sb")
            nc.sync.dma_start(pack_sb[:], pack[:])
            ag_sb = cpool.tile([P, AGW], i16, tag="ag_sb")
            nc.sync.dma_start(ag_sb[:], agidx[:])

            def pcol(key, j0, nw, dt=None):
                ap = pack_sb[:, OFF[key] + j0 : OFF[key] + j0 + nw]
                return ap.bitcast(dt) if dt is not None else ap

            def pcol_p(key, j0, nw, np_, dt=None):
                ap = pack_sb[:np_, OFF[key] + j0 : OFF[key] + j0 + nw]
                return ap.bitcast(dt) if dt is not None else ap

            # weights (small, gate the level chains)
            w_ru_sb = cpool.tile([H, H2], f32, tag="w_ru")
            nc.sync.dma_start(w_ru_sb[:], W_ru[:])
            w_c_sb = cpool.tile([H, H], f32, tag="w_c")
            nc.sync.dma_start(w_c_sb[:], W_c[:])

            # L0 embedding gathers head the gpsimd queue
            g_cat = []
            gw = H2 + H if with_h0 else H2
            gsrc = P_cat if with_h0 else P_zc
            for c in range(NCH):
                t = spool.tile([P, gw], f32, tag="g_cat", bufs=NCH, name="g_cat")
                nc.gpsimd.indirect_dma_start(
                    out=t[:],
                    out_offset=None,
                    in_=gsrc[:],
                    in_offset=bass.IndirectOffsetOnAxis(
                        ap=pcol("items", c, 1, i32), axis=0
                    ),
                )
                g_cat.append(t)
            if with_h0:
                h0_idx_sb = cpool.tile([P, NCH], i32, tag="h0_idx_sb")
                nc.sync.dma_start(h0_idx_sb[:], h0_idx[:])
                g_h0 = []
                for c in range(NCH):
                    g = spool.tile([P, H], f32, tag="g_h0", bufs=NCH, name="g_h0")
                    nc.gpsimd.indirect_dma_start(
                        out=g[:],
                        out_offset=None,
                        in_=h0c[:],
                        in_offset=bass.IndirectOffsetOnAxis(
                            ap=h0_idx_sb[:, c : c + 1], axis=0
                        ),
                    )
                    g_h0.append(g)
            # per-level embedding gathers (prefetched; only need idx)
            lvl_emb = {}
            for k in range(1, kmax):
                n = nk[k]
                J = (n + P - 1) // P
                embs = []
                for j in range(J):
                    j0 = j * P
                    nj = min(P, n - j0)
                    e_cat = spool.tile(
                        [P, H2 + H], f32, tag="e_cat", bufs=2 * kmax, name="e_cat"
                    )
                    nc.gpsimd.indirect_dma_start(
                        out=e_cat[:nj, :],
                        out_offset=None,
                        in_=P_cat[:],
                        in_offset=bass.IndirectOffsetOnAxis(
                            ap=pcol_p(f"idx{k}", j, 1, nj, i32), axis=0
                        ),
                    )
                    embs.append(e_cat)
                lvl_emb[k] = embs

            # bulk ws load, gated behind the LAST embedding gather via an
            # explicit WAW dep (a 1-elem dummy write into each ws tile): the
            # latency-critical gathers get the DMA engines to themselves,
            # then ws streams in under the compute-only level chains.
            gate_src = lvl_emb[kmax - 1][-1] if kmax > 1 else g_cat[-1]
            gate_bf = cpool.tile([P, 1], bf16, tag="gate_bf")
            nc.vector.tensor_copy(gate_bf[:, :1], gate_src[:, :1])
            ws_sb = []
            for i, (v0, w) in enumerate(ws_splits):
                t = cpool.tile([H, w], bf16, tag=f"ws{i}", name=f"ws{i}")
                nc.sync.dma_start(t[:1, :1], gate_bf[:1, :1])
                nc.sync.dma_start(t[:], ws[:, v0 : v0 + w])
                ws_sb.append(t)

            # helper tiles (gpsimd queue, after the gathers)
            ident = cpool.tile([P, P], f32, tag="ident")
            make_identity(nc, ident[:])

            # transposed all-states tile: cols [0:512) hold the level-0 h of
            # every pair; cols [AOFF[k], AOFF[k]+nk[k]) hold level k's outputs
            # in compact order.  Every h_prev fetch and the final hT assembly
            # is then a single gpsimd ap_gather over a column prefix.
            allT = cpool.tile([H, AW], f32, tag="allT", name="allT")
            hT = [
                cpool.tile([H, P], bf16, tag=f"hT{c}", name=f"hT{c}")
                for c in range(NCH)
            ]

            # ---------- level 0: all 512 pairs -> allT[:, 0:512)
            if not with_h0:
                # h0 == 0: z = sigmoid(P_z[i]), c = tanh(P_c[i]), h = c - z*c
                # elementwise in natural layout, then one transpose per chunk
                for c in range(NCH):
                    z_nat = spool.tile([P, H], f32, tag="z_nat", bufs=2, name="z_nat")
                    nc.scalar.activation(z_nat[:], g_cat[c][:, 0:H], AF.Sigmoid)
                    c_nat = spool.tile([P, H], f32, tag="c_nat", bufs=2, name="c_nat")
                    nc.scalar.activation(c_nat[:], g_cat[c][:, H:H2], AF.Tanh)
                    h_tmp = spool.tile([P, H], f32, tag="h_tmp", bufs=2, name="h_tmp")
                    nc.vector.tensor_mul(h_tmp[:], z_nat[:], c_nat[:])
                    nc.vector.tensor_sub(h_tmp[:], c_nat[:], h_tmp[:])
                    ps = spsum.tile([P, P], f32, tag="tr_ps", bufs=2, name="tr_ps")
                    nc.tensor.transpose(ps[:], h_tmp[:], ident[:])
                    nc.vector.tensor_copy(allT[:, c * P : (c + 1) * P], ps[:])
            else:
                zT = cpool.tile([H, N], f32, tag="zT")
                cT = cpool.tile([H, N], f32, tag="cT")
                z_ps = spsum.tile([H, N], f32, tag="z_ps2", name="z_ps")
                c_ps = spsum.tile([H, N], f32, tag="c_ps2", name="c_ps")
                hp_ps = spsum.tile([H, N], f32, tag="hp_ps", name="hp_ps")
                for c in range(NCH):
                    nc.tensor.matmul(
                        hp_ps[:, c * P : (c + 1) * P],
                        g_h0[c][:],
                        ident[:],
                        is_transpose=True,
                        start=(c == 0),
                        stop=(c == NCH - 1),
                    )
                hprevT0 = cpool.tile([H, N], f32, tag="hprevT0")
                nc.vector.tensor_copy(hprevT0[:], hp_ps[:])

                r_ps = spsum.tile([H, N], f32, tag="r_ps", name="r_ps")
                for c in range(NCH):
                    nc.tensor.matmul(
                        r_ps[:, c * P : (c + 1) * P],
                        g_cat[c][:, 0:H],
                        ident[:],
                        is_transpose=True,
                        start=(c == 0),
                        stop=False,
                    )
                    nc.tensor.matmul(
                        z_ps[:, c * P : (c + 1) * P],
                        g_cat[c][:, H:H2],
                        ident[:],
                        is_transpose=True,
                        start=(c == 0),
                        stop=False,
                    )
                nc.tensor.matmul(
                    r_ps[:], w_ru_sb[:, 0:H], hprevT0[:], start=False, stop=True
                )
                nc.tensor.matmul(
                    z_ps[:], w_ru_sb[:, H:H2], hprevT0[:], start=False, stop=True
                )
                rT = cpool.tile([H, N], f32, tag="rT0")
                nc.scalar.activation(rT[:], r_ps[:], AF.Sigmoid)
                nc.scalar.activation(zT[:], z_ps[:], AF.Sigmoid)
                rh = cpool.tile([H, N], f32, tag="rh0")
                nc.vector.tensor_mul(rh[:], rT[:], hprevT0[:])
                for c in range(NCH):
                    nc.tensor.matmul(
                        c_ps[:, c * P : (c + 1) * P],
                        g_cat[c][:, H2 : H2 + H],
                        ident[:],
                        is_transpose=True,
                        start=(c == 0),
                        stop=False,
                    )
                nc.tensor.matmul(c_ps[:], w_c_sb[:], rh[:], start=False, stop=True)
                nc.scalar.activation(cT[:], c_ps[:], AF.Tanh)
                # h = c + z*(hprev - c)
                nc.vector.tensor_sub(allT[:, :N], hprevT0[:], cT[:])
                nc.vector.tensor_mul(allT[:, :N], zT[:], allT[:, :N])
                nc.vector.tensor_add(allT[:, :N], cT[:], allT[:, :N])

            # ---------- levels 1..kmax-1 (compact, padded size nk[k])
            for k in range(1, kmax):
                n = nk[k]
                J = (n + P - 1) // P
                a0 = AOFF[k]
                o_ag, w_ag, nj4 = AGOFF[("lvl", k)]
                # one ap_gather fetches every pair's h_prev for this level
                # from the already-written allT prefix [0, a0)
                hprevT = spool.tile([H, _pad4(n)], f32, tag="hprevT", name="hprevT")
                nc.gpsimd.ap_gather(
                    out_ap=hprevT[:, :nj4],
                    in_ap=allT[:, :a0],
                    idxs_ap=ag_sb[:, o_ag : o_ag + w_ag],
                    channels=P,
                    num_elems=a0,
                    d=1,
                    num_idxs=nj4,
                )
                for j in range(J):
                    j0 = j * P
                    nj = min(P, n - j0)
                    e_cat = lvl_emb[k][j]
                    hp = hprevT[:, j0 : j0 + nj]

                    # GRU math; embeddings (biases pre-folded) enter via
                    # transpose-matmuls (emitted first in each group)
                    r_ps = spsum.tile([H, P], f32, tag="r_ps", name="r_ps")
                    nc.tensor.matmul(
                        r_ps[:, :nj],
                        e_cat[:nj, 0:H],
                        ident[:nj, :nj],
                        is_transpose=True,
                        start=True,
                        stop=False,
                    )
                    nc.tensor.matmul(
                        r_ps[:, :nj], w_ru_sb[:, 0:H], hp, start=False, stop=True
                    )
                    rT = spool.tile([H, P], f32, tag="rT_l", name="rT")
                    nc.scalar.activation(rT[:, :nj], r_ps[:, :nj], AF.Sigmoid)
                    z_ps2 = spsum.tile([H, P], f32, tag="z_ps2", name="z_ps2")
                    nc.tensor.matmul(
                        z_ps2[:, :nj],
                        e_cat[:nj, H:H2],
                        ident[:nj, :nj],
                        is_transpose=True,
                        start=True,
                        stop=False,
                    )
                    nc.tensor.matmul(
                        z_ps2[:, :nj], w_ru_sb[:, H:H2], hp, start=False, stop=True
                    )
                    zTl = spool.tile([H, P], f32, tag="zT_l", name="zTl")
                    nc.scalar.activation(zTl[:, :nj], z_ps2[:, :nj], AF.Sigmoid)
                    # 1-z = sigmoid(-x); both z muls run before tanh lands
                    zmT = spool.tile([H, P], f32, tag="zm_l", name="zmT")
                    nc.scalar.activation(
                        zmT[:, :nj], z_ps2[:, :nj], AF.Sigmoid, scale=-1.0
                    )
                    rh = spool.tile([H, P], f32, tag="rh_l", name="rh")
                    nc.vector.tensor_mul(rh[:, :nj], rT[:, :nj], hp)
                    zh = spool.tile([H, P], f32, tag="zh_l", name="zh")
                    nc.vector.tensor_mul(zh[:, :nj], zTl[:, :nj], hp)
                    c_ps2 = spsum.tile([H, P], f32, tag="c_ps2", name="c_ps2")
                    nc.tensor.matmul(
                        c_ps2[:, :nj],
                        e_cat[:nj, H2 : H2 + H],
                        ident[:nj, :nj],
                        is_transpose=True,
                        start=True,
                        stop=False,
                    )
                    nc.tensor.matmul(
                        c_ps2[:, :nj],
                        w_c_sb[:],
                        rh[:, :nj],
                        start=False,
                        stop=True,
                    )
                    cTl = spool.tile([H, P], f32, tag="cT_l", name="cTl")
                    nc.scalar.activation(cTl[:, :nj], c_ps2[:, :nj], AF.Tanh)
                    # h_new = z*hprev + (1-z)*c, written straight into this
                    # level's allT block; z*hprev precomputed above so only
                    # two DVE ops sit after the tanh on the chain
                    av = allT[:, a0 + j0 : a0 + j0 + nj]
                    nc.vector.tensor_mul(av, zmT[:, :nj], cTl[:, :nj])
                    nc.vector.tensor_add(av, zh[:, :nj], av)

            # ---------- final transposed state: one ap_gather per chunk
            # picks each pair's own output column out of allT
            for c in range(NCH):
                o_ag, w_ag, _ = AGOFF[("fin", c)]
                hf = spool.tile([H, P], f32, tag="hT_f", name="hT_f")
                nc.gpsimd.ap_gather(
                    out_ap=hf[:, :P],
                    in_ap=allT[:, :AW],
                    idxs_ap=ag_sb[:, o_ag : o_ag + w_ag],
                    channels=P,
                    num_elems=AW,
                    d=1,
                    num_idxs=P,
                )
                nc.vector.tensor_copy(hT[c][:], hf[:])


        # ---------- big projection: logits[128c : 128c+128, :] = hT[c].T @ ws
        with (
            tc.tile_pool(name="big", bufs=4) as bpool,
            tc.tile_pool(name="big_ps", bufs=4, space="PSUM") as bpsum,
        ):
            cp = 0
            dq = 0
            for v0 in range(0, V, STG_CHUNK):
                w = min(STG_CHUNK, V - v0)
                wsi, off = divmod(v0, WS_CHUNK)
                for c in range(NCH):
                    stage = bpool.tile([P, STG_CHUNK], bf16, tag="stage", name="stage")
                    for s0 in range(0, w, PS_N):
                        pw = min(PS_N, w - s0)
                        o_ps = bpsum.tile([P, PS_N], f32, tag="o_ps", name="o_ps")
                        for m0 in range(0, pw, MM_N):
                            mw = min(MM_N, pw - m0)
                            rr = ws_sb[wsi][:, off + s0 + m0 : off + s0 + m0 + mw]
                            nc.tensor.matmul(
                                o_ps[:, m0 : m0 + mw],
                                hT[c][:],
                                rr,
                                start=True,
                                stop=True,
                            )
                        # PSUM→SBUF (+bf16 cast): only DVE/Act can read PSUM
                        if cp % 2 == 0:
                            nc.vector.tensor_copy(stage[:, s0 : s0 + pw], o_ps[:, :pw])
                        else:
                            nc.scalar.copy(stage[:, s0 : s0 + pw], o_ps[:, :pw])
                        cp += 1
                    out_ap = logits[c * P : (c + 1) * P, v0 : v0 + w]
                    if dq % 3 == 0:
                        nc.sync.dma_start(out_ap, stage[:, :w])
                    elif dq % 3 == 1:
                        nc.scalar.dma_start(out_ap, stage[:, :w])
                    else:
                        nc.gpsimd.dma_start(out_ap, stage[:, :w])
                    dq += 1

    nc.finalize()
    return nc


_PROGRAM_CACHE = {}


def kernel(users, items, h0, P_ru, W_ru, b_ru, P_c, W_c, b_c, ws):
    _install_ntff_hook()
    from concourse.bass_utils import run_bass_kernel_spmd

    users = np.asarray(users)
    items = np.asarray(items)
    h0 = np.asarray(h0, dtype=np.float32)
    with_h0 = bool(np.any(h0))

    per_core, kmax, nk = _build_core_data(users, items, h0, with_h0)

    key = (kmax, tuple(nk), with_h0)
    if key not in _PROGRAM_CACHE:
        _PROGRAM_CACHE[key] = _build_program(kmax, nk, with_h0)
    nc = _PROGRAM_CACHE[key]

    # biases folded into the embedding tables (each gathered row always adds
    # exactly P[i] + b, so P := P + b is equivalent)
    P_cat = np.concatenate(
        [
            np.asarray(P_ru, dtype=np.float32) + np.asarray(b_ru, np.float32)[None, :],
            np.asarray(P_c, dtype=np.float32) + np.asarray(b_c, np.float32)[None, :],
        ],
        axis=1,
    )
    shared = {
        "P_cat": P_cat,
        "P_zc": np.ascontiguousarray(P_cat[:, H:]),
        "W_ru": np.ascontiguousarray(W_ru, dtype=np.float32),
        "W_c": np.ascontiguousarray(W_c, dtype=np.float32),
        "ws": np.ascontiguousarray(
            np.asarray(ws, dtype=np.float32).astype(ml_dtypes.bfloat16)
        ),
    }
    in_maps = [{**shared, **per_core[c]} for c in range(NC)]

    res = run_bass_kernel_spmd(nc, in_maps, core_ids=list(range(NC)), trace=TRACE)
    _LAST_RESULTS["exec_time_ns"] = res.exec_time_ns
    _LAST_RESULTS["mean_exec_time_ns"] = res.mean_exec_time_ns
    _LAST_RESULTS["trace"] = res.instructions_and_trace
    _LAST_RESULTS["profile_json"] = res.profile_json

    return np.concatenate(
        [np.asarray(res.results[c]["logits"]) for c in range(NC)], axis=0
    ).astype(np.float32)

